# revision 24
# baseline (speedup 1.0000x reference)
"""Trainium2 Bass kernel for nn_Cross_Attention (dual cross channel-attention block).

Architecture (8 NeuronCores, data-parallel):
  core i -> (batch b = i//2, row-half h = i%2) of the 4x[64,256,256] images.

Math restructuring (exact, up to float assoc):
  qkv = dwconv3x3(conv1x1(x, W))  is computed with the 3x3 depthwise conv
  *folded* into the 1x1 conv. Two taps are packed per matmul (K=128) using
  partition-duplicated input tensors whose bottom half is the same image
  shifted by one row / one column, so each qk row costs 5 matmuls per image
  instead of 9.
  Channel attention needs only second moments of q,k:
     S_a[c,d] = sum_p qb[c,p] ka[d,p],  S_b[c,d] = sum_p qa[c,p] kb[d,p]
     n_*[c]   = sum_p q[c,p]^2
  computed on-chip (Gram via PE-transposed bf16 operands + PE matmuls,
  norms via ScalarE Square+accum), then AllReduce'd across the 2 cores
  sharing a batch. Softmax + all downstream linear layers are folded into
  12 per-batch [128,128] bf16 pair-stationaries applied in one output pass
  that computes TWO output rows per PSUM tile (top/bottom 64 partitions);
  results are DMA'd straight from PSUM to DRAM.
"""

import os
import sys

sys.path.insert(0, "/opt/trn_rl_repo")

import ml_dtypes
import numpy as np

import concourse.bass as bass
import concourse.bacc as bacc
import concourse.tile as tile
from concourse import mybir
from concourse.bass_utils import run_bass_kernel_spmd
from concourse.masks import make_identity

F32 = mybir.dt.float32
BF16 = mybir.dt.bfloat16

B, C, H, W = 4, 64, 256, 256
HEADS, CH = 8, 8
WP = W + 2          # zero-padded width
N_CORES = 8
R_LOC = H // 2      # output rows per core
BLK = 16            # rows per streaming block
TAPS = [(dy, dx) for dy in (-1, 0, 1) for dx in (-1, 0, 1)]
GROUPS = [[0, 1], [2, 3], [4, 5], [6, 7]]


def kernel_body(tc, outs, ins, cfg):
    nc = tc.nc
    rows = cfg["rows"]
    blk = cfg["blk"]
    nblk = rows // blk
    w = cfg["w"]
    wp = w + 2
    groups = cfg["groups"]
    nch_blk = blk * w // 128  # 128-px transpose chunks per block

    xy = ins["xy"]            # [128, rows+2, wp] dram bf16 (x on 0:64, y on 64:128)
    out_d = outs["out"]       # [64, rows, w] dram f32

    from contextlib import ExitStack

    with ExitStack() as ctx:
        consts = ctx.enter_context(tc.tile_pool(name="consts", bufs=1))
        stats = ctx.enter_context(tc.tile_pool(name="stats", bufs=1))
        small = ctx.enter_context(tc.tile_pool(name="small", bufs=2))
        xin2 = ctx.enter_context(tc.tile_pool(name="xin2", bufs=3))
        dram = ctx.enter_context(tc.tile_pool(name="dram", bufs=1, space="DRAM"))
        psg = ctx.enter_context(tc.tile_pool(name="psg", bufs=1, space="PSUM"))

        # ---- constants ----
        wpa_t = consts.tile([128, 4, 128], BF16)
        nc.sync.dma_start(wpa_t, ins["wpa"])
        wpb_t = consts.tile([128, 4, 128], BF16)
        nc.sync.dma_start(wpb_t, ins["wpb"])
        wsg_t = consts.tile([128, 2, 128], BF16)
        nc.sync.dma_start(wsg_t, ins["wsg"])
        wva_t = consts.tile([64, 64], F32)
        nc.sync.dma_start(wva_t, ins["wva"])
        wvb_t = consts.tile([64, 64], F32)
        nc.sync.dma_start(wvb_t, ins["wvb"])
        w1t_t = consts.tile([64, 64], F32)
        nc.sync.dma_start(w1t_t, ins["w1t"])
        w2t_t = consts.tile([64, 64], F32)
        nc.sync.dma_start(w2t_t, ins["w2t"])
        catcb_t = consts.tile([128, 64], F32)
        nc.sync.dma_start(catcb_t, ins["catcb"])
        dwva_t = consts.tile([64, 9], F32)
        nc.sync.dma_start(dwva_t, ins["dwva"])
        dwvb_t = consts.tile([64, 9], F32)
        nc.sync.dma_start(dwvb_t, ins["dwvb"])
        tva_t = consts.tile([64, 1], F32)
        nc.sync.dma_start(tva_t, ins["tva"])
        tvb_t = consts.tile([64, 1], F32)
        nc.sync.dma_start(tvb_t, ins["tvb"])
        hmask_t = consts.tile([64, 64], F32)
        nc.sync.dma_start(hmask_t, ins["hmask"])
        ident = consts.tile([128, 128], F32)
        make_identity(nc, ident)
        ident_bf = consts.tile([128, 128], BF16)
        make_identity(nc, ident_bf)
        s2p = consts.tile([128, 12, 128], BF16)  # pass-2 pair stationaries

        # ---- stats accumulator: [A.B gram | A.A gram | B.B gram] ----
        gram_ps = psg.tile([128, 384], F32)

        # ================= PASS 1: qk + stats =================
        with ExitStack() as c1:
            dup = c1.enter_context(tc.tile_pool(name="dup", bufs=2))
            qkev = c1.enter_context(tc.tile_pool(name="qkev", bufs=2))
            qkt = c1.enter_context(tc.tile_pool(name="qkt", bufs=3))
            psA = c1.enter_context(tc.tile_pool(name="psA", bufs=3, space="PSUM"))
            psB = c1.enter_context(tc.tile_pool(name="psB", bufs=2, space="PSUM"))

            for b in range(nblk):
                B0 = b * blk
                # partition-duplicated inputs: top 64 = image, bottom 64 =
                # image shifted one row (d1) / one column (d2).
                xxd1 = dup.tile([128, blk, wp], BF16, tag="xxd1")
                nc.sync.dma_start(xxd1[0:64], xy[0:64, B0 : B0 + blk, :])
                nc.sync.dma_start(xxd1[64:128], xy[0:64, B0 + 1 : B0 + blk + 1, :])
                yyd1 = dup.tile([128, blk, wp], BF16, tag="yyd1")
                nc.sync.dma_start(yyd1[0:64], xy[64:128, B0 : B0 + blk, :])
                nc.sync.dma_start(yyd1[64:128], xy[64:128, B0 + 1 : B0 + blk + 1, :])
                xxd2 = dup.tile([128, blk, wp], BF16, tag="xxd2")
                nc.sync.dma_start(xxd2[0:64], xy[0:64, B0 + 2 : B0 + blk + 2, :])
                nc.sync.dma_start(xxd2[64:128, :, 0 : wp - 1],
                                  xy[0:64, B0 + 2 : B0 + blk + 2, 1:wp])
                yyd2 = dup.tile([128, blk, wp], BF16, tag="yyd2")
                nc.sync.dma_start(yyd2[0:64], xy[64:128, B0 + 2 : B0 + blk + 2, :])
                nc.sync.dma_start(yyd2[64:128, :, 0 : wp - 1],
                                  xy[64:128, B0 + 2 : B0 + blk + 2, 1:wp])

                qa_bf = qkev.tile([128, blk, w], BF16, tag="qa")
                qb_bf = qkev.tile([128, blk, w], BF16, tag="qb")
                for l in range(blk):
                    row = B0 + l
                    pAB = psA.tile([128, 512], F32, tag="pAB")
                    pA = pAB[:, 0:w]
                    pB = pAB[:, w : 2 * w]
                    # A side: pairs (dy=-1,dy=0) x 3dx from d1; pair
                    # ((1,-1),(1,0)) from d2; single (1,1) from d2 bottom.
                    nc.tensor.matmul(pA, lhsT=wpa_t[:, 0, :],
                                     rhs=xxd1[:, l, 0:w], start=True, stop=False)
                    nc.tensor.matmul(pA, lhsT=wpa_t[:, 1, :],
                                     rhs=xxd1[:, l, 1 : w + 1], start=False, stop=False)
                    nc.tensor.matmul(pA, lhsT=wpa_t[:, 2, :],
                                     rhs=xxd1[:, l, 2 : w + 2], start=False, stop=False)
                    nc.tensor.matmul(pA, lhsT=wpa_t[:, 3, :],
                                     rhs=xxd2[:, l, 0:w], start=False, stop=False)
                    nc.tensor.matmul(pA, lhsT=wsg_t[64:128, 0, :],
                                     rhs=xxd2[64:128, l, 1 : w + 1],
                                     start=False, stop=True)
                    nc.tensor.matmul(pB, lhsT=wpb_t[:, 0, :],
                                     rhs=yyd1[:, l, 0:w], start=True, stop=False)
                    nc.tensor.matmul(pB, lhsT=wpb_t[:, 1, :],
                                     rhs=yyd1[:, l, 1 : w + 1], start=False, stop=False)
                    nc.tensor.matmul(pB, lhsT=wpb_t[:, 2, :],
                                     rhs=yyd1[:, l, 2 : w + 2], start=False, stop=False)
                    nc.tensor.matmul(pB, lhsT=wpb_t[:, 3, :],
                                     rhs=yyd2[:, l, 0:w], start=False, stop=False)
                    nc.tensor.matmul(pB, lhsT=wsg_t[64:128, 1, :],
                                     rhs=yyd2[64:128, l, 1 : w + 1],
                                     start=False, stop=True)
                    # evacuate to bf16 for the Gram
                    nc.vector.tensor_copy(qa_bf[:, l, :], pA)
                    nc.vector.tensor_copy(qb_bf[:, l, :], pB)
                # blocked transpose via PE (bf16) + per-chunk Gram accumulate.
                # qT layout: [qbT | qaT] so one matmul covers A.B and A.A.
                qa_fl = qa_bf.rearrange("p a b -> p (a b)")
                qb_fl = qb_bf.rearrange("p a b -> p (a b)")
                for cc in range(nch_blk):
                    pst = psB.tile([128, 256], BF16, tag="tp")
                    nc.tensor.transpose(pst[:, 0:128],
                                        qb_fl[:, cc * 128 : (cc + 1) * 128],
                                        ident_bf)
                    nc.tensor.transpose(pst[:, 128:256],
                                        qa_fl[:, cc * 128 : (cc + 1) * 128],
                                        ident_bf)
                    qT = qkt.tile([128, 256], BF16, tag="qT")
                    if cc % 2 == 0:
                        nc.vector.tensor_copy(qT, pst)
                    else:
                        nc.scalar.copy(qT, pst)
                    first = b == 0 and cc == 0
                    last = b == nblk - 1 and cc == nch_blk - 1
                    nc.tensor.matmul(
                        gram_ps[:, 0:256],
                        lhsT=qT[:, 128:256],
                        rhs=qT,
                        start=first,
                        stop=last,
                    )
                    nc.tensor.matmul(
                        gram_ps[:, 256:384],
                        lhsT=qT[:, 0:128],
                        rhs=qT[:, 0:128],
                        start=first,
                        stop=last,
                    )

            # ---- finalize + allreduce stats ----
            # self-gram diagonals -> per-channel sumsq norms
            stpack = stats.tile([128, 130], F32)
            nc.vector.tensor_copy(stpack[:, 0:128], gram_ps[:, 0:128])
            dtmp = stats.tile([128, 128], F32)
            nc.vector.tensor_mul(dtmp, gram_ps[:, 128:256], ident)
            nc.vector.tensor_reduce(stpack[:, 128:129], dtmp,
                                    axis=mybir.AxisListType.X,
                                    op=mybir.AluOpType.add)
            nc.vector.tensor_mul(dtmp, gram_ps[:, 256:384], ident)
            nc.vector.tensor_reduce(stpack[:, 129:130], dtmp,
                                    axis=mybir.AxisListType.X,
                                    op=mybir.AluOpType.add)
            bounce_in = dram.tile([128, 130], F32)
            bounce_out = dram.tile([128, 130], F32)
            nc.sync.dma_start(bounce_in, stpack)
            nc.gpsimd.collective_compute(
                "AllReduce",
                mybir.AluOpType.add,
                replica_groups=groups,
                ins=[bounce_in.opt()],
                outs=[bounce_out.opt()],
            )
            stall = stats.tile([128, 130], F32)
            nc.sync.dma_start(stall, bounce_out)
            if "dbg" in outs:
                nc.sync.dma_start(outs["dbg"], stall)

            # ---- softmax + fold (tiny) ----
            # stall[:, 0:128] = Gram out[chA, chB]; chA rows = (qa 0:64 | ka
            # 64:128), chB cols = (qb 0:64 | kb 64:128).
            #   S_b  = stall[0:64, 64:128]   (qa . kb)  rows=qa
            #   S_aT = stall[64:128, 0:64]   (ka . qb)  rows=ka
            # col 128 = img-A sumsq (qa|ka), col 129 = img-B sumsq (qb|kb)
            rn = stats.tile([128, 2], F32)
            nc.scalar.activation(rn, stall[:, 128:130],
                                 mybir.ActivationFunctionType.Sqrt)
            nc.vector.reciprocal(rn, rn)

            ident64 = ident[0:64, 0:64]

            def softmax_bd(scores_full, name):
                # scores_full: [64,64] sbuf; per-head block-diag softmax -> [64,8]
                masked = stats.tile([64, 64], F32, tag=f"masked_{name}")
                nc.vector.tensor_mul(masked, scores_full, hmask_t)
                sbd = stats.tile([64, 8], F32, tag=f"sbd_{name}")
                nc.vector.tensor_copy(sbd, masked[:, 0:8])
                for h in range(1, HEADS):
                    nc.vector.tensor_add(sbd, sbd, masked[:, h * 8 : (h + 1) * 8])
                mx = stats.tile([64, 1], F32, tag=f"mx_{name}")
                se = stats.tile([64, 1], F32, tag=f"se_{name}")
                nc.vector.tensor_reduce(mx, sbd, axis=mybir.AxisListType.X,
                                        op=mybir.AluOpType.max)
                nc.vector.tensor_scalar_sub(sbd, sbd, mx)
                nc.scalar.activation(sbd, sbd, mybir.ActivationFunctionType.Exp,
                                     accum_out=se)
                nc.vector.reciprocal(se, se)
                nc.vector.tensor_scalar_mul(sbd, sbd, se)
                return sbd

            # scores_a: transpose S_aT -> [qb, ka]; scale rows(ka) first,
            # then rows(qb)
            sa_t = stats.tile([64, 64], F32)
            nc.vector.tensor_scalar_mul(sa_t, stall[64:128, 0:64], rn[64:128, 0:1])
            paT = psB.tile([64, 64], F32, tag="sm")
            nc.tensor.transpose(paT, sa_t, ident64)
            rqa_scale = stats.tile([64, 1], F32)
            nc.vector.tensor_mul(rqa_scale, rn[0:64, 1:2], tva_t)  # rn_qb * temp
            sa_full = stats.tile([64, 64], F32)
            nc.vector.tensor_scalar_mul(sa_full, paT, rqa_scale)
            attn_a = softmax_bd(sa_full, "a")

            # scores_b: S_b rows=qa; col-scale by rn_kb via double transpose
            sbT = psB.tile([64, 64], F32, tag="sm")
            nc.tensor.transpose(sbT, stall[0:64, 64:128], ident64)
            sb_t = stats.tile([64, 64], F32)
            nc.vector.tensor_scalar_mul(sb_t, sbT, rn[64:128, 1:2])  # rows kb
            sb_ps = psB.tile([64, 64], F32, tag="sm")
            nc.tensor.transpose(sb_ps, sb_t, ident64)
            rqb_scale = stats.tile([64, 1], F32)
            nc.vector.tensor_mul(rqb_scale, rn[0:64, 0:1], tvb_t)  # rn_qa * (-temp)
            sb_full = stats.tile([64, 64], F32)
            nc.vector.tensor_scalar_mul(sb_full, sb_ps, rqb_scale)
            attn_b = softmax_bd(sb_full, "b")

            # fold: per-tap stationaries S(dy,dx) = s2[:, t, :]
            s2 = stats.tile([128, 9, 64], F32)

            def fold_side(attn, w1t_c, wv_c, dwv_c, prow, name):
                bd = stats.tile([64, 64], F32, tag=f"bd_{name}")
                for h in range(HEADS):
                    nc.vector.tensor_copy(bd[:, h * 8 : (h + 1) * 8], attn)
                nc.vector.tensor_mul(bd, bd, hmask_t)
                m_ps = psB.tile([64, 64], F32, tag="sm")
                nc.tensor.matmul(m_ps, lhsT=w1t_c, rhs=bd, start=True, stop=True)
                m_sb = stats.tile([64, 64], F32, tag=f"msb_{name}")
                nc.vector.tensor_copy(m_sb, m_ps)
                mT_ps = psB.tile([64, 64], F32, tag="sm")
                nc.tensor.transpose(mT_ps, m_sb, ident64)
                mT = stats.tile([64, 64], F32, tag=f"mT_{name}")
                nc.vector.tensor_copy(mT, mT_ps)  # [d, o]
                for t in range(9):
                    tmp = small.tile([64, 64], F32, tag=f"tmp_{name}")
                    nc.vector.tensor_scalar_mul(tmp, mT, dwv_c[:, t : t + 1])
                    s2ps = psB.tile([64, 64], F32, tag="sm")
                    nc.tensor.matmul(s2ps, lhsT=wv_c, rhs=tmp, start=True,
                                     stop=True)
                    nc.vector.tensor_copy(s2[prow : prow + 64, t, :], s2ps)

            fold_side(attn_a, w1t_t, wva_t, dwva_t, 0, "a")
            fold_side(attn_b, w2t_t, wvb_t, dwvb_t, 64, "b")
            # merge the concat/residual path into the (0,0) tap (t=4)
            nc.vector.tensor_add(s2[:, 4, :], s2[:, 4, :], catcb_t)

            # compose 12 pair-stationaries [xych, 128]: variant v = r*3+dx,
            # r in 0..3 = rhs xy-row offset within the out pair.
            #  cols 0:64 -> out row 2m,  cols 64:128 -> out row 2m+1
            nc.vector.memset(s2p, 0.0)
            for dx in range(3):
                nc.vector.tensor_copy(s2p[:, 0 * 3 + dx, 0:64], s2[:, 0 + dx, :])
                nc.vector.tensor_copy(s2p[:, 1 * 3 + dx, 0:64], s2[:, 3 + dx, :])
                nc.vector.tensor_copy(s2p[:, 1 * 3 + dx, 64:128], s2[:, 0 + dx, :])
                nc.vector.tensor_copy(s2p[:, 2 * 3 + dx, 0:64], s2[:, 6 + dx, :])
                nc.vector.tensor_copy(s2p[:, 2 * 3 + dx, 64:128], s2[:, 3 + dx, :])
                nc.vector.tensor_copy(s2p[:, 3 * 3 + dx, 64:128], s2[:, 6 + dx, :])

        # ================= PASS 2: output =================
        # out rows viewed as (pair, parity): psum pair tiles hold row 2m on
        # partitions 0:64 and row 2m+1 on partitions 64:128.
        out_r = out_d.rearrange("c (m two) w -> c m two w", two=2)
        with ExitStack() as c2:
            obuf = c2.enter_context(tc.tile_pool(name="obuf", bufs=2))
            pso = c2.enter_context(tc.tile_pool(name="pso", bufs=6, space="PSUM"))
            for b in range(nblk):
                B0 = b * blk
                xt2 = xin2.tile([128, blk + 2, wp], BF16, tag="xt2")
                nc.sync.dma_start(xt2, xy[:, B0 : B0 + blk + 2, :])
                ob2 = obuf.tile([128, blk // 2, w], BF16, tag="ob2")
                for mi in range(blk // 2):
                    po = pso.tile([128, 256], F32, tag="po")
                    for r in range(4):
                        for dx in range(3):
                            nc.tensor.matmul(
                                po,
                                lhsT=s2p[:, r * 3 + dx, :],
                                rhs=xt2[:, 2 * mi + r, dx : dx + w],
                                start=(r == 0 and dx == 0),
                                stop=(r == 3 and dx == 2),
                            )
                    if mi % 2 == 0:
                        nc.vector.tensor_copy(ob2[:, mi, :], po)
                    else:
                        nc.scalar.copy(ob2[:, mi, :], po)
                mp = b * (blk // 2)
                nc.sync.dma_start(out_r[:, mp : mp + blk // 2, 0, :],
                                  ob2[0:64, :, :])
                nc.sync.dma_start(out_r[:, mp : mp + blk // 2, 1, :],
                                  ob2[64:128, :, :])


# ---------------------------------------------------------------------------
# host side
# ---------------------------------------------------------------------------

def prep_weights(inputs):
    f = lambda k: np.asarray(inputs[k], np.float32)
    qkv_A_w, qkv_B_w = f("qkv_A_w"), f("qkv_B_w")
    dw_A, dw_B = f("dw_A_w")[:, 0], f("dw_B_w")[:, 0]    # [192, 3, 3]
    proj_A, proj_B = f("proj_A_w"), f("proj_B_w")
    concat = f("concat_w")
    temp = f("temperature").reshape(HEADS)

    def tap_w(qkv_w, dw, dy, dx):
        # [64 in, 128 out] = W_qk^T scaled by the dwconv tap
        wqk = qkv_w[:128]            # [128, 64]
        return (wqk * dw[:128, dy + 1, dx + 1][:, None]).T

    def pack_pairs(qkv_w, dw):
        wp = np.zeros((128, 4, 128), np.float32)
        for p, dx in enumerate((-1, 0, 1)):
            wp[0:64, p, :] = tap_w(qkv_w, dw, -1, dx)
            wp[64:128, p, :] = tap_w(qkv_w, dw, 0, dx)
        wp[0:64, 3, :] = tap_w(qkv_w, dw, 1, -1)
        wp[64:128, 3, :] = tap_w(qkv_w, dw, 1, 0)
        return wp

    wsg = np.zeros((128, 2, 128), np.float32)
    wsg[64:128, 0, :] = tap_w(qkv_A_w, dw_A, 1, 1)
    wsg[64:128, 1, :] = tap_w(qkv_B_w, dw_B, 1, 1)

    CA, CB = concat[:, :64], concat[:, 64:]
    bf = lambda a: np.ascontiguousarray(a).astype(ml_dtypes.bfloat16)
    consts = {
        "wpa": bf(pack_pairs(qkv_A_w, dw_A)),
        "wpb": bf(pack_pairs(qkv_B_w, dw_B)),
        "wsg": bf(wsg),
        "wva": np.ascontiguousarray(qkv_A_w[128:192]),   # [d, xc]
        "wvb": np.ascontiguousarray(qkv_B_w[128:192]),
        "w1t": np.ascontiguousarray((CA @ proj_A).T),
        "w2t": np.ascontiguousarray((CB @ proj_B).T),
        "catcb": np.ascontiguousarray(np.concatenate([CA.T, CB.T], axis=0)),
        "dwva": np.ascontiguousarray(dw_A[128:192].reshape(64, 9)),
        "dwvb": np.ascontiguousarray(dw_B[128:192].reshape(64, 9)),
        "tva": np.repeat(temp, CH).reshape(64, 1).astype(np.float32),
        "tvb": (-np.repeat(temp, CH)).reshape(64, 1).astype(np.float32),
        "hmask": np.kron(np.eye(HEADS, dtype=np.float32),
                         np.ones((CH, CH), np.float32)),
    }
    return consts


def shard_inputs(inputs):
    x = np.asarray(inputs["x"], np.float32)
    y = np.asarray(inputs["y"], np.float32)
    b, c, h, w = x.shape
    xp = np.zeros((b, c, h + 2, w + 2), np.float32)
    yp = np.zeros((b, c, h + 2, w + 2), np.float32)
    xp[:, :, 1 : h + 1, 1 : w + 1] = x
    yp[:, :, 1 : h + 1, 1 : w + 1] = y
    consts = prep_weights(inputs)
    in_maps = []
    rloc = h // 2
    for core in range(N_CORES):
        bi, half = core // 2, core % 2
        r0 = half * rloc
        xy = np.concatenate(
            [xp[bi, :, r0 : r0 + rloc + 2, :], yp[bi, :, r0 : r0 + rloc + 2, :]],
            axis=0,
        )
        m = {"xy": np.ascontiguousarray(xy.astype(ml_dtypes.bfloat16))}
        m.update(consts)
        in_maps.append(m)
    return in_maps


_CACHE = {}


def build_program(cfg):
    key = (cfg["rows"], cfg["blk"], cfg["w"], len(cfg["groups"]))
    if key in _CACHE:
        return _CACHE[key]
    nc = bacc.Bacc("TRN2", target_bir_lowering=False, debug=False,
                   num_devices=cfg["n_cores"])
    rows, w = cfg["rows"], cfg["w"]
    ins = {
        "xy": nc.dram_tensor("xy", [128, rows + 2, w + 2], BF16,
                             kind="ExternalInput").ap(),
        "wpa": nc.dram_tensor("wpa", [128, 4, 128], BF16,
                              kind="ExternalInput").ap(),
        "wpb": nc.dram_tensor("wpb", [128, 4, 128], BF16,
                              kind="ExternalInput").ap(),
        "wsg": nc.dram_tensor("wsg", [128, 2, 128], BF16,
                              kind="ExternalInput").ap(),
        "wva": nc.dram_tensor("wva", [64, 64], F32, kind="ExternalInput").ap(),
        "wvb": nc.dram_tensor("wvb", [64, 64], F32, kind="ExternalInput").ap(),
        "w1t": nc.dram_tensor("w1t", [64, 64], F32, kind="ExternalInput").ap(),
        "w2t": nc.dram_tensor("w2t", [64, 64], F32, kind="ExternalInput").ap(),
        "catcb": nc.dram_tensor("catcb", [128, 64], F32,
                                kind="ExternalInput").ap(),
        "dwva": nc.dram_tensor("dwva", [64, 9], F32, kind="ExternalInput").ap(),
        "dwvb": nc.dram_tensor("dwvb", [64, 9], F32, kind="ExternalInput").ap(),
        "tva": nc.dram_tensor("tva", [64, 1], F32, kind="ExternalInput").ap(),
        "tvb": nc.dram_tensor("tvb", [64, 1], F32, kind="ExternalInput").ap(),
        "hmask": nc.dram_tensor("hmask", [64, 64], F32,
                                kind="ExternalInput").ap(),
    }
    outs = {
        "out": nc.dram_tensor("out", [64, rows, w], BF16,
                              kind="ExternalOutput").ap(),
    }
    with tile.TileContext(nc) as tc:
        kernel_body(tc, outs, ins, cfg)
    nc.compile()
    _CACHE[key] = nc
    return nc


def default_cfg():
    return {
        "rows": R_LOC,
        "blk": BLK,
        "w": W,
        "n_cores": N_CORES,
        "groups": GROUPS,
    }


def _run(inputs, trace=False):
    cfg = default_cfg()
    nc = build_program(cfg)
    in_maps = shard_inputs(inputs)
    res = run_bass_kernel_spmd(nc, in_maps, core_ids=list(range(N_CORES)),
                               trace=trace)
    x = np.asarray(inputs["x"])
    b, c, h, w = x.shape
    out = np.empty((b, c, h, w), np.float32)
    rloc = h // 2
    for core in range(N_CORES):
        bi, half = core // 2, core % 2
        out[bi, :, half * rloc : (half + 1) * rloc, :] = np.asarray(
            res.results[core]["out"]).astype(np.float32)
    return out, res


def kernel(**inputs):
    out, _ = _run(inputs, trace=False)
    return out


# revision 29
# speedup vs baseline: 1.0875x; 1.0875x over previous
"""Trainium2 Bass kernel for nn_Cross_Attention (dual cross channel-attention block).

Architecture (8 NeuronCores, data-parallel):
  core i -> (batch b = i//2, row-half h = i%2) of the 4x[64,256,256] images.

Math restructuring (exact, up to float assoc):
  qkv = dwconv3x3(conv1x1(x, W))  is computed with the 3x3 depthwise conv
  *folded* into the 1x1 conv. Two taps are packed per matmul (K=128) using
  partition-duplicated input tensors whose bottom half is the same image
  shifted by one row / one column, so each qk row costs 5 matmuls per image
  instead of 9.
  Channel attention needs only second moments of q,k:
     S_a[c,d] = sum_p qb[c,p] ka[d,p],  S_b[c,d] = sum_p qa[c,p] kb[d,p]
     n_*[c]   = sum_p q[c,p]^2
  computed on-chip (Gram via PE-transposed bf16 operands + PE matmuls,
  norms via ScalarE Square+accum), then AllReduce'd across the 2 cores
  sharing a batch. Softmax + all downstream linear layers are folded into
  12 per-batch [128,128] bf16 pair-stationaries applied in one output pass
  that computes TWO output rows per PSUM tile (top/bottom 64 partitions);
  results are DMA'd straight from PSUM to DRAM.
"""

import os
import sys

sys.path.insert(0, "/opt/trn_rl_repo")

import ml_dtypes
import numpy as np

import concourse.bass as bass
import concourse.bacc as bacc
import concourse.tile as tile
from concourse import mybir
from concourse.bass_utils import run_bass_kernel_spmd
from concourse.masks import make_identity

F32 = mybir.dt.float32
BF16 = mybir.dt.bfloat16

B, C, H, W = 4, 64, 256, 256
HEADS, CH = 8, 8
WP = W + 2          # zero-padded width
N_CORES = 8
R_LOC = H // 2      # output rows per core
BLK = 16            # rows per streaming block
TAPS = [(dy, dx) for dy in (-1, 0, 1) for dx in (-1, 0, 1)]
GROUPS = [[0, 1], [2, 3], [4, 5], [6, 7]]


def kernel_body(tc, outs, ins, cfg):
    nc = tc.nc
    rows = cfg["rows"]
    blk = cfg["blk"]
    nblk = rows // blk
    w = cfg["w"]
    wp = w + 2
    groups = cfg["groups"]
    nch_blk = blk * w // 128  # 128-px transpose chunks per block

    xy = ins["xy"]            # [128, rows+2, wp] dram bf16 (x on 0:64, y on 64:128)
    out_d = outs["out"]       # [64, rows, w] dram f32

    from contextlib import ExitStack

    with ExitStack() as ctx:
        consts = ctx.enter_context(tc.tile_pool(name="consts", bufs=1))
        stats = ctx.enter_context(tc.tile_pool(name="stats", bufs=1))
        small = ctx.enter_context(tc.tile_pool(name="small", bufs=2))
        xin2 = ctx.enter_context(tc.tile_pool(name="xin2", bufs=3))
        dram = ctx.enter_context(tc.tile_pool(name="dram", bufs=1, space="DRAM"))
        psg = ctx.enter_context(tc.tile_pool(name="psg", bufs=1, space="PSUM"))

        # ---- constants ----
        wpa_t = consts.tile([128, 4, 128], BF16)
        nc.sync.dma_start(wpa_t, ins["wpa"])
        wpb_t = consts.tile([128, 4, 128], BF16)
        nc.sync.dma_start(wpb_t, ins["wpb"])
        wsg_t = consts.tile([128, 2, 128], BF16)
        nc.sync.dma_start(wsg_t, ins["wsg"])
        wva_t = consts.tile([64, 64], F32)
        nc.sync.dma_start(wva_t, ins["wva"])
        wvb_t = consts.tile([64, 64], F32)
        nc.sync.dma_start(wvb_t, ins["wvb"])
        w1t_t = consts.tile([64, 64], F32)
        nc.sync.dma_start(w1t_t, ins["w1t"])
        w2t_t = consts.tile([64, 64], F32)
        nc.sync.dma_start(w2t_t, ins["w2t"])
        catcb_t = consts.tile([128, 64], F32)
        nc.sync.dma_start(catcb_t, ins["catcb"])
        dwva_t = consts.tile([64, 9], F32)
        nc.sync.dma_start(dwva_t, ins["dwva"])
        dwvb_t = consts.tile([64, 9], F32)
        nc.sync.dma_start(dwvb_t, ins["dwvb"])
        tva_t = consts.tile([64, 1], F32)
        nc.sync.dma_start(tva_t, ins["tva"])
        tvb_t = consts.tile([64, 1], F32)
        nc.sync.dma_start(tvb_t, ins["tvb"])
        hmask_t = consts.tile([64, 64], F32)
        nc.sync.dma_start(hmask_t, ins["hmask"])
        ident = consts.tile([128, 128], F32)
        make_identity(nc, ident)
        ident_bf = consts.tile([128, 128], BF16)
        make_identity(nc, ident_bf)
        s2p = consts.tile([128, 12, 128], BF16)  # pass-2 pair stationaries

        # ---- stats accumulator: [A.B gram | A.A gram | B.B gram] ----
        gram_ps = psg.tile([128, 384], F32)

        # ================= PASS 1: qk + stats =================
        # first block split in two so compute starts after a smaller DMA
        blocks = [(0, blk // 2), (blk // 2, blk // 2)] + [
            (b0, blk) for b0 in range(blk, rows, blk)
        ]
        with ExitStack() as c1:
            dup = c1.enter_context(tc.tile_pool(name="dup", bufs=2))
            qkev = c1.enter_context(tc.tile_pool(name="qkev", bufs=2))
            qkt = c1.enter_context(tc.tile_pool(name="qkt", bufs=3))
            psA = c1.enter_context(tc.tile_pool(name="psA", bufs=2, space="PSUM"))
            psB = c1.enter_context(tc.tile_pool(name="psB", bufs=2, space="PSUM"))

            for bi_, (B0, nr) in enumerate(blocks):
                # partition-duplicated inputs: top 64 = image, bottom 64 =
                # image shifted one row (d1) / one column (d2).
                xxd1 = dup.tile([128, nr, wp], BF16, tag="xxd1",
                                padded_shape=[128, blk, wp])
                nc.sync.dma_start(xxd1[0:64], xy[0:64, B0 : B0 + nr, :])
                nc.sync.dma_start(xxd1[64:128], xy[0:64, B0 + 1 : B0 + nr + 1, :])
                yyd1 = dup.tile([128, nr, wp], BF16, tag="yyd1",
                                padded_shape=[128, blk, wp])
                nc.sync.dma_start(yyd1[0:64], xy[64:128, B0 : B0 + nr, :])
                nc.sync.dma_start(yyd1[64:128], xy[64:128, B0 + 1 : B0 + nr + 1, :])
                xxd2 = dup.tile([128, nr, wp], BF16, tag="xxd2",
                                padded_shape=[128, blk, wp])
                nc.sync.dma_start(xxd2[0:64], xy[0:64, B0 + 2 : B0 + nr + 2, :])
                nc.sync.dma_start(xxd2[64:128, :, 0 : wp - 1],
                                  xy[0:64, B0 + 2 : B0 + nr + 2, 1:wp])
                yyd2 = dup.tile([128, nr, wp], BF16, tag="yyd2",
                                padded_shape=[128, blk, wp])
                nc.sync.dma_start(yyd2[0:64], xy[64:128, B0 + 2 : B0 + nr + 2, :])
                nc.sync.dma_start(yyd2[64:128, :, 0 : wp - 1],
                                  xy[64:128, B0 + 2 : B0 + nr + 2, 1:wp])

                qa_bf = qkev.tile([128, nr, w], BF16, tag="qa",
                                  padded_shape=[128, blk, w])
                qb_bf = qkev.tile([128, nr, w], BF16, tag="qb",
                                  padded_shape=[128, blk, w])
                for l in range(0, nr, 2):
                    # two output rows per matmul group: N=512
                    pA = psA.tile([128, 512], F32, tag="pA2")
                    pB = psA.tile([128, 512], F32, tag="pB2")
                    # A side: pairs (dy=-1,dy=0) x 3dx from d1; pair
                    # ((1,-1),(1,0)) from d2; single (1,1) from d2 bottom.
                    nc.tensor.matmul(pA, lhsT=wpa_t[:, 0, :],
                                     rhs=xxd1[:, l : l + 2, 0:w],
                                     start=True, stop=False)
                    nc.tensor.matmul(pA, lhsT=wpa_t[:, 1, :],
                                     rhs=xxd1[:, l : l + 2, 1 : w + 1],
                                     start=False, stop=False)
                    nc.tensor.matmul(pA, lhsT=wpa_t[:, 2, :],
                                     rhs=xxd1[:, l : l + 2, 2 : w + 2],
                                     start=False, stop=False)
                    nc.tensor.matmul(pA, lhsT=wpa_t[:, 3, :],
                                     rhs=xxd2[:, l : l + 2, 0:w],
                                     start=False, stop=False)
                    nc.tensor.matmul(pA, lhsT=wsg_t[64:128, 0, :],
                                     rhs=xxd2[64:128, l : l + 2, 1 : w + 1],
                                     start=False, stop=True)
                    nc.tensor.matmul(pB, lhsT=wpb_t[:, 0, :],
                                     rhs=yyd1[:, l : l + 2, 0:w],
                                     start=True, stop=False)
                    nc.tensor.matmul(pB, lhsT=wpb_t[:, 1, :],
                                     rhs=yyd1[:, l : l + 2, 1 : w + 1],
                                     start=False, stop=False)
                    nc.tensor.matmul(pB, lhsT=wpb_t[:, 2, :],
                                     rhs=yyd1[:, l : l + 2, 2 : w + 2],
                                     start=False, stop=False)
                    nc.tensor.matmul(pB, lhsT=wpb_t[:, 3, :],
                                     rhs=yyd2[:, l : l + 2, 0:w],
                                     start=False, stop=False)
                    nc.tensor.matmul(pB, lhsT=wsg_t[64:128, 1, :],
                                     rhs=yyd2[64:128, l : l + 2, 1 : w + 1],
                                     start=False, stop=True)
                    # evacuate to bf16 for the Gram, split across engines
                    qa2 = qa_bf[:, l : l + 2, :].rearrange("p a b -> p (a b)")
                    qb2 = qb_bf[:, l : l + 2, :].rearrange("p a b -> p (a b)")
                    if l % 4 == 0:
                        nc.vector.tensor_copy(qa2, pA)
                        nc.scalar.copy(qb2, pB)
                    else:
                        nc.scalar.copy(qa2, pA)
                        nc.vector.tensor_copy(qb2, pB)
                # blocked transpose via PE (bf16) + per-chunk Gram accumulate.
                # qT layout: [qbT | qaT] so one matmul covers A.B and A.A.
                qa_fl = qa_bf.rearrange("p a b -> p (a b)")
                qb_fl = qb_bf.rearrange("p a b -> p (a b)")
                nch = nr * w // 128
                for cc in range(nch):
                    pst = psB.tile([128, 256], BF16, tag="tp")
                    nc.tensor.transpose(pst[:, 0:128],
                                        qb_fl[:, cc * 128 : (cc + 1) * 128],
                                        ident_bf)
                    nc.tensor.transpose(pst[:, 128:256],
                                        qa_fl[:, cc * 128 : (cc + 1) * 128],
                                        ident_bf)
                    qT = qkt.tile([128, 256], BF16, tag="qT")
                    if cc % 2 == 0:
                        nc.vector.tensor_copy(qT, pst)
                    else:
                        nc.scalar.copy(qT, pst)
                    first = bi_ == 0 and cc == 0
                    last = bi_ == len(blocks) - 1 and cc == nch - 1
                    nc.tensor.matmul(
                        gram_ps[:, 0:256],
                        lhsT=qT[:, 128:256],
                        rhs=qT,
                        start=first,
                        stop=last,
                    )
                    nc.tensor.matmul(
                        gram_ps[:, 256:384],
                        lhsT=qT[:, 0:128],
                        rhs=qT[:, 0:128],
                        start=first,
                        stop=last,
                    )

            # ---- finalize + allreduce stats ----
            # self-gram diagonals -> per-channel sumsq norms
            stpack = stats.tile([128, 130], F32)
            nc.vector.tensor_copy(stpack[:, 0:128], gram_ps[:, 0:128])
            dtmp = stats.tile([128, 128], F32)
            nc.vector.tensor_mul(dtmp, gram_ps[:, 128:256], ident)
            nc.vector.tensor_reduce(stpack[:, 128:129], dtmp,
                                    axis=mybir.AxisListType.X,
                                    op=mybir.AluOpType.add)
            nc.vector.tensor_mul(dtmp, gram_ps[:, 256:384], ident)
            nc.vector.tensor_reduce(stpack[:, 129:130], dtmp,
                                    axis=mybir.AxisListType.X,
                                    op=mybir.AluOpType.add)
            bounce_in = dram.tile([128, 130], F32)
            bounce_out = dram.tile([128, 130], F32)
            nc.sync.dma_start(bounce_in, stpack)
            nc.gpsimd.collective_compute(
                "AllReduce",
                mybir.AluOpType.add,
                replica_groups=groups,
                ins=[bounce_in.opt()],
                outs=[bounce_out.opt()],
            )
            stall = stats.tile([128, 130], F32)
            nc.sync.dma_start(stall, bounce_out)
            if "dbg" in outs:
                nc.sync.dma_start(outs["dbg"], stall)

            # ---- softmax + fold (tiny) ----
            # stall[:, 0:128] = Gram out[chA, chB]; chA rows = (qa 0:64 | ka
            # 64:128), chB cols = (qb 0:64 | kb 64:128).
            #   S_b  = stall[0:64, 64:128]   (qa . kb)  rows=qa
            #   S_aT = stall[64:128, 0:64]   (ka . qb)  rows=ka
            # col 128 = img-A sumsq (qa|ka), col 129 = img-B sumsq (qb|kb)
            rn = stats.tile([128, 2], F32)
            nc.scalar.activation(rn, stall[:, 128:130],
                                 mybir.ActivationFunctionType.Sqrt)
            nc.vector.reciprocal(rn, rn)

            ident64 = ident[0:64, 0:64]

            def softmax_bd(scores_full, name):
                # scores_full: [64,64] sbuf; per-head block-diag softmax -> [64,8]
                masked = stats.tile([64, 64], F32, tag=f"masked_{name}")
                nc.vector.tensor_mul(masked, scores_full, hmask_t)
                sbd = stats.tile([64, 8], F32, tag=f"sbd_{name}")
                nc.vector.tensor_copy(sbd, masked[:, 0:8])
                for h in range(1, HEADS):
                    nc.vector.tensor_add(sbd, sbd, masked[:, h * 8 : (h + 1) * 8])
                mx = stats.tile([64, 1], F32, tag=f"mx_{name}")
                se = stats.tile([64, 1], F32, tag=f"se_{name}")
                nc.vector.tensor_reduce(mx, sbd, axis=mybir.AxisListType.X,
                                        op=mybir.AluOpType.max)
                nc.vector.tensor_scalar_sub(sbd, sbd, mx)
                nc.scalar.activation(sbd, sbd, mybir.ActivationFunctionType.Exp,
                                     accum_out=se)
                nc.vector.reciprocal(se, se)
                nc.vector.tensor_scalar_mul(sbd, sbd, se)
                return sbd

            # scores_a: transpose S_aT -> [qb, ka]; scale rows(ka) first,
            # then rows(qb)
            sa_t = stats.tile([64, 64], F32)
            nc.vector.tensor_scalar_mul(sa_t, stall[64:128, 0:64], rn[64:128, 0:1])
            paT = psB.tile([64, 64], F32, tag="sm", bufs=1)
            nc.tensor.transpose(paT, sa_t, ident64)
            rqa_scale = stats.tile([64, 1], F32)
            nc.vector.tensor_mul(rqa_scale, rn[0:64, 1:2], tva_t)  # rn_qb * temp
            sa_full = stats.tile([64, 64], F32)
            nc.vector.tensor_scalar_mul(sa_full, paT, rqa_scale)
            attn_a = softmax_bd(sa_full, "a")

            # scores_b: S_b rows=qa; col-scale by rn_kb via double transpose
            sbT = psB.tile([64, 64], F32, tag="sm", bufs=1)
            nc.tensor.transpose(sbT, stall[0:64, 64:128], ident64)
            sb_t = stats.tile([64, 64], F32)
            nc.vector.tensor_scalar_mul(sb_t, sbT, rn[64:128, 1:2])  # rows kb
            sb_ps = psB.tile([64, 64], F32, tag="sm", bufs=1)
            nc.tensor.transpose(sb_ps, sb_t, ident64)
            rqb_scale = stats.tile([64, 1], F32)
            nc.vector.tensor_mul(rqb_scale, rn[0:64, 0:1], tvb_t)  # rn_qa * (-temp)
            sb_full = stats.tile([64, 64], F32)
            nc.vector.tensor_scalar_mul(sb_full, sb_ps, rqb_scale)
            attn_b = softmax_bd(sb_full, "b")

            # fold: per-tap stationaries S(dy,dx) = s2[:, t, :]
            s2 = stats.tile([128, 9, 64], F32)

            def fold_side(attn, w1t_c, wv_c, dwv_c, prow, name):
                bd = stats.tile([64, 64], F32, tag=f"bd_{name}")
                for h in range(HEADS):
                    nc.vector.tensor_copy(bd[:, h * 8 : (h + 1) * 8], attn)
                nc.vector.tensor_mul(bd, bd, hmask_t)
                m_ps = psB.tile([64, 64], F32, tag="sm", bufs=1)
                nc.tensor.matmul(m_ps, lhsT=w1t_c, rhs=bd, start=True, stop=True)
                m_sb = stats.tile([64, 64], F32, tag=f"msb_{name}")
                nc.vector.tensor_copy(m_sb, m_ps)
                mT_ps = psB.tile([64, 64], F32, tag="sm", bufs=1)
                nc.tensor.transpose(mT_ps, m_sb, ident64)
                mT = stats.tile([64, 64], F32, tag=f"mT_{name}")
                nc.vector.tensor_copy(mT, mT_ps)  # [d, o]
                for t in range(9):
                    tmp = small.tile([64, 64], F32, tag=f"tmp_{name}")
                    nc.vector.tensor_scalar_mul(tmp, mT, dwv_c[:, t : t + 1])
                    s2ps = psB.tile([64, 64], F32, tag="sm", bufs=1)
                    nc.tensor.matmul(s2ps, lhsT=wv_c, rhs=tmp, start=True,
                                     stop=True)
                    nc.vector.tensor_copy(s2[prow : prow + 64, t, :], s2ps)

            fold_side(attn_a, w1t_t, wva_t, dwva_t, 0, "a")
            fold_side(attn_b, w2t_t, wvb_t, dwvb_t, 64, "b")
            # merge the concat/residual path into the (0,0) tap (t=4)
            nc.vector.tensor_add(s2[:, 4, :], s2[:, 4, :], catcb_t)

            # compose 12 pair-stationaries [xych, 128]: variant v = r*3+dx,
            # r in 0..3 = rhs xy-row offset within the out pair.
            #  cols 0:64 -> out row 2m,  cols 64:128 -> out row 2m+1
            nc.vector.memset(s2p, 0.0)
            nc.vector.tensor_copy(s2p[:, 0:3, 0:64], s2[:, 0:3, :])
            nc.vector.tensor_copy(s2p[:, 3:6, 0:64], s2[:, 3:6, :])
            nc.vector.tensor_copy(s2p[:, 3:6, 64:128], s2[:, 0:3, :])
            nc.vector.tensor_copy(s2p[:, 6:9, 0:64], s2[:, 6:9, :])
            nc.vector.tensor_copy(s2p[:, 6:9, 64:128], s2[:, 3:6, :])
            nc.vector.tensor_copy(s2p[:, 9:12, 64:128], s2[:, 6:9, :])

        # ================= PASS 2: output =================
        # out rows viewed as (pair, parity): psum pair tiles hold row 2m on
        # partitions 0:64 and row 2m+1 on partitions 64:128.
        out_r = out_d.rearrange("c (m two) w -> c m two w", two=2)
        with ExitStack() as c2:
            obuf = c2.enter_context(tc.tile_pool(name="obuf", bufs=2))
            pso = c2.enter_context(tc.tile_pool(name="pso", bufs=6, space="PSUM"))
            for b in range(nblk):
                B0 = b * blk
                xt2 = xin2.tile([128, blk + 2, wp], BF16, tag="xt2")
                nc.sync.dma_start(xt2, xy[:, B0 : B0 + blk + 2, :])
                # even/odd row view: [p, parity, h, w] addresses rows (2h+par)
                # so a [.., h:h+2, ..] slice spans rows q and q+2 -> N=512
                # matmuls covering two out-pairs (4 out rows) at once.
                xt2v = xt2.rearrange("p (h two) w -> p two h w", two=2)
                ob2 = obuf.tile([128, blk // 2, w], BF16, tag="ob2")
                for mi in range(0, blk // 2, 2):
                    po = pso.tile([128, 512], F32, tag="po")
                    for r in range(4):
                        q = 2 * mi + r
                        for dx in range(3):
                            nc.tensor.matmul(
                                po,
                                lhsT=s2p[:, r * 3 + dx, :],
                                rhs=xt2v[:, q % 2, q // 2 : q // 2 + 2,
                                         dx : dx + w],
                                start=(r == 0 and dx == 0),
                                stop=(r == 3 and dx == 2),
                            )
                    ob22 = ob2[:, mi : mi + 2, :].rearrange("p a b -> p (a b)")
                    if mi % 4 == 0:
                        nc.vector.tensor_copy(ob22, po)
                    else:
                        nc.scalar.copy(ob22, po)
                mp = b * (blk // 2)
                nc.sync.dma_start(out_r[:, mp : mp + blk // 2, 0, :],
                                  ob2[0:64, :, :])
                nc.sync.dma_start(out_r[:, mp : mp + blk // 2, 1, :],
                                  ob2[64:128, :, :])


# ---------------------------------------------------------------------------
# host side
# ---------------------------------------------------------------------------

def prep_weights(inputs):
    f = lambda k: np.asarray(inputs[k], np.float32)
    qkv_A_w, qkv_B_w = f("qkv_A_w"), f("qkv_B_w")
    dw_A, dw_B = f("dw_A_w")[:, 0], f("dw_B_w")[:, 0]    # [192, 3, 3]
    proj_A, proj_B = f("proj_A_w"), f("proj_B_w")
    concat = f("concat_w")
    temp = f("temperature").reshape(HEADS)

    def tap_w(qkv_w, dw, dy, dx):
        # [64 in, 128 out] = W_qk^T scaled by the dwconv tap
        wqk = qkv_w[:128]            # [128, 64]
        return (wqk * dw[:128, dy + 1, dx + 1][:, None]).T

    def pack_pairs(qkv_w, dw):
        wp = np.zeros((128, 4, 128), np.float32)
        for p, dx in enumerate((-1, 0, 1)):
            wp[0:64, p, :] = tap_w(qkv_w, dw, -1, dx)
            wp[64:128, p, :] = tap_w(qkv_w, dw, 0, dx)
        wp[0:64, 3, :] = tap_w(qkv_w, dw, 1, -1)
        wp[64:128, 3, :] = tap_w(qkv_w, dw, 1, 0)
        return wp

    wsg = np.zeros((128, 2, 128), np.float32)
    wsg[64:128, 0, :] = tap_w(qkv_A_w, dw_A, 1, 1)
    wsg[64:128, 1, :] = tap_w(qkv_B_w, dw_B, 1, 1)

    CA, CB = concat[:, :64], concat[:, 64:]
    bf = lambda a: np.ascontiguousarray(a).astype(ml_dtypes.bfloat16)
    consts = {
        "wpa": bf(pack_pairs(qkv_A_w, dw_A)),
        "wpb": bf(pack_pairs(qkv_B_w, dw_B)),
        "wsg": bf(wsg),
        "wva": np.ascontiguousarray(qkv_A_w[128:192]),   # [d, xc]
        "wvb": np.ascontiguousarray(qkv_B_w[128:192]),
        "w1t": np.ascontiguousarray((CA @ proj_A).T),
        "w2t": np.ascontiguousarray((CB @ proj_B).T),
        "catcb": np.ascontiguousarray(np.concatenate([CA.T, CB.T], axis=0)),
        "dwva": np.ascontiguousarray(dw_A[128:192].reshape(64, 9)),
        "dwvb": np.ascontiguousarray(dw_B[128:192].reshape(64, 9)),
        "tva": np.repeat(temp, CH).reshape(64, 1).astype(np.float32),
        "tvb": (-np.repeat(temp, CH)).reshape(64, 1).astype(np.float32),
        "hmask": np.kron(np.eye(HEADS, dtype=np.float32),
                         np.ones((CH, CH), np.float32)),
    }
    return consts


def shard_inputs(inputs):
    x = np.asarray(inputs["x"], np.float32)
    y = np.asarray(inputs["y"], np.float32)
    b, c, h, w = x.shape
    xp = np.zeros((b, c, h + 2, w + 2), np.float32)
    yp = np.zeros((b, c, h + 2, w + 2), np.float32)
    xp[:, :, 1 : h + 1, 1 : w + 1] = x
    yp[:, :, 1 : h + 1, 1 : w + 1] = y
    consts = prep_weights(inputs)
    in_maps = []
    rloc = h // 2
    for core in range(N_CORES):
        bi, half = core // 2, core % 2
        r0 = half * rloc
        xy = np.concatenate(
            [xp[bi, :, r0 : r0 + rloc + 2, :], yp[bi, :, r0 : r0 + rloc + 2, :]],
            axis=0,
        )
        m = {"xy": np.ascontiguousarray(xy.astype(ml_dtypes.bfloat16))}
        m.update(consts)
        in_maps.append(m)
    return in_maps


_CACHE = {}


def build_program(cfg):
    key = (cfg["rows"], cfg["blk"], cfg["w"], len(cfg["groups"]))
    if key in _CACHE:
        return _CACHE[key]
    nc = bacc.Bacc("TRN2", target_bir_lowering=False, debug=False,
                   num_devices=cfg["n_cores"])
    rows, w = cfg["rows"], cfg["w"]
    ins = {
        "xy": nc.dram_tensor("xy", [128, rows + 2, w + 2], BF16,
                             kind="ExternalInput").ap(),
        "wpa": nc.dram_tensor("wpa", [128, 4, 128], BF16,
                              kind="ExternalInput").ap(),
        "wpb": nc.dram_tensor("wpb", [128, 4, 128], BF16,
                              kind="ExternalInput").ap(),
        "wsg": nc.dram_tensor("wsg", [128, 2, 128], BF16,
                              kind="ExternalInput").ap(),
        "wva": nc.dram_tensor("wva", [64, 64], F32, kind="ExternalInput").ap(),
        "wvb": nc.dram_tensor("wvb", [64, 64], F32, kind="ExternalInput").ap(),
        "w1t": nc.dram_tensor("w1t", [64, 64], F32, kind="ExternalInput").ap(),
        "w2t": nc.dram_tensor("w2t", [64, 64], F32, kind="ExternalInput").ap(),
        "catcb": nc.dram_tensor("catcb", [128, 64], F32,
                                kind="ExternalInput").ap(),
        "dwva": nc.dram_tensor("dwva", [64, 9], F32, kind="ExternalInput").ap(),
        "dwvb": nc.dram_tensor("dwvb", [64, 9], F32, kind="ExternalInput").ap(),
        "tva": nc.dram_tensor("tva", [64, 1], F32, kind="ExternalInput").ap(),
        "tvb": nc.dram_tensor("tvb", [64, 1], F32, kind="ExternalInput").ap(),
        "hmask": nc.dram_tensor("hmask", [64, 64], F32,
                                kind="ExternalInput").ap(),
    }
    outs = {
        "out": nc.dram_tensor("out", [64, rows, w], BF16,
                              kind="ExternalOutput").ap(),
    }
    with tile.TileContext(nc) as tc:
        kernel_body(tc, outs, ins, cfg)
    nc.compile()
    _CACHE[key] = nc
    return nc


def default_cfg():
    return {
        "rows": R_LOC,
        "blk": BLK,
        "w": W,
        "n_cores": N_CORES,
        "groups": GROUPS,
    }


def _run(inputs, trace=False):
    cfg = default_cfg()
    nc = build_program(cfg)
    in_maps = shard_inputs(inputs)
    res = run_bass_kernel_spmd(nc, in_maps, core_ids=list(range(N_CORES)),
                               trace=trace)
    x = np.asarray(inputs["x"])
    b, c, h, w = x.shape
    out = np.empty((b, c, h, w), np.float32)
    rloc = h // 2
    for core in range(N_CORES):
        bi, half = core // 2, core % 2
        out[bi, :, half * rloc : (half + 1) * rloc, :] = np.asarray(
            res.results[core]["out"]).astype(np.float32)
    return out, res


def kernel(**inputs):
    out, _ = _run(inputs, trace=False)
    return out
